# revision 1
# baseline (speedup 1.0000x reference)
"""Trainium2 Bass kernel for gated multi-head attention + residual + LayerNorm.

Problem (nn_CNP_5669356834854):
    B=2, L=2048, D=1024, H=16, DK=DV=64
    Q = q@wq.T+bq; K = k@wk.T+bk; V = v@wv.T+bv   (per-head split)
    attn = softmax((Q K^T / sqrt(DK)) * k_gate  [masked])
    out = LayerNorm(attn @ V @ wo.T + bo + q)

Sharding: 8 cores = (batch b in {0,1}) x (head-group hg in {0..3}, 4 heads each).
Launch 1 computes normalized per-head attention outputs O^T per core.
Launch 2 shards (batch, 512-row chunk) for the output projection + residual + LN.

Everything is computed in "T-space" (transposed layouts) so that no on-chip
transposes are needed:
    S^T[lk,lq] = (K^T)^T-free matmul with lhsT=K^T tile, rhs=Q^T
    P^T = exp(S^T * gate^T - 20)        (the -20 cancels in normalization)
    O_aug = [V | ones64]^T-matmul: rows 0:64 = unnormalized O^T, rows 64:128 =
            the softmax denominator replicated across 64 partitions (free
            broadcast), so normalization is one reciprocal + one multiply.
"""

import numpy as np
import ml_dtypes

import concourse.bacc as bacc
import concourse.tile as tile
from concourse import mybir
from concourse.bass_utils import run_bass_kernel_spmd

B, L, D, H, DK, DV = 2, 2048, 1024, 16, 64, 64
EPS = 1e-5
NCORE = 8
HPC = 4  # heads per core
NKC = D // 128  # 8 contraction chunks
NLKT = L // 128  # 16 lk tiles
NCH = 4  # lq chunks
CH = L // NCH  # 512
MPC = HPC * DK  # 256 projected rows per core
EXP_BIAS = -20.0

F32 = mybir.dt.float32
BF16 = mybir.dt.bfloat16
NPBF16 = ml_dtypes.bfloat16
AF = mybir.ActivationFunctionType


def _bf(x):
    return np.ascontiguousarray(x).astype(NPBF16)


def _kc_layout(a):
    """[D, N] -> [128, NKC, N] with row r = kc*128+p  ->  [p, kc, :]."""
    d, n = a.shape
    assert d == NKC * 128
    return np.ascontiguousarray(a.reshape(NKC, 128, n).transpose(1, 0, 2))


def build_l1(masked: bool, use_bq: bool, use_bk: bool, use_bv: bool):
    nc = bacc.Bacc("TRN2", target_bir_lowering=False)

    qT = nc.declare_dram_parameter("qT", [128, NKC, L], BF16, isOutput=False)
    kT = nc.declare_dram_parameter("kT", [128, NKC, L], BF16, isOutput=False)
    vT = nc.declare_dram_parameter("vT", [128, NKC, L], BF16, isOutput=False)
    wqT = nc.declare_dram_parameter("wqT", [128, NKC, MPC], BF16, isOutput=False)
    wkT = nc.declare_dram_parameter("wkT", [128, NKC, MPC], BF16, isOutput=False)
    wvT = nc.declare_dram_parameter("wvT", [128, NKC, MPC], BF16, isOutput=False)
    gT = nc.declare_dram_parameter("gT", [HPC, L, L], BF16, isOutput=False)
    if use_bq:
        bqP = nc.declare_dram_parameter("bqP", [128, 2], F32, isOutput=False)
    if use_bk:
        bkP = nc.declare_dram_parameter("bkP", [128, 2], F32, isOutput=False)
    if use_bv:
        bvR = nc.declare_dram_parameter("bvR", [1, MPC], F32, isOutput=False)
    if masked:
        mbT = nc.declare_dram_parameter("mbT", [L, L], BF16, isOutput=False)
    oT = nc.declare_dram_parameter("oT", [128, 2, L], BF16, isOutput=True)

    HF = L // 2

    with tile.TileContext(nc) as tc:
        with (
            tc.tile_pool(name="xs", bufs=2) as xs,
            tc.tile_pool(name="ws", bufs=1) as ws,
            tc.tile_pool(name="qk", bufs=1) as qk,
            tc.tile_pool(name="gp", bufs=4) as gp,
            tc.tile_pool(name="tp", bufs=2) as tp,
            tc.tile_pool(name="pp", bufs=6) as pp,
            tc.tile_pool(name="op", bufs=1) as opl,
            tc.tile_pool(name="rp", bufs=2) as rp,
            tc.tile_pool(name="ps_s", bufs=1, space="PSUM") as ps_s,
            tc.tile_pool(name="ps_o", bufs=2, space="PSUM") as ps_o,
        ):
            wq_sb = ws.tile([128, NKC, MPC], BF16, tag="wq")
            nc.sync.dma_start(out=wq_sb, in_=wqT[:, :, :])
            wk_sb = ws.tile([128, NKC, MPC], BF16, tag="wk")
            nc.sync.dma_start(out=wk_sb, in_=wkT[:, :, :])
            wv_sb = ws.tile([128, NKC, MPC], BF16, tag="wv")
            nc.sync.dma_start(out=wv_sb, in_=wvT[:, :, :])

            QT = qk.tile([128, 2, L], BF16, tag="qt")
            KT = qk.tile([128, 2, L], BF16, tag="kt")
            Vaug = qk.tile([128, NLKT, HPC, 128], BF16, tag="va")
            nc.vector.memset(Vaug[:, :, :, 64:128], 1.0)
            ebias = ws.tile([128, 1], F32, tag="eb")
            nc.vector.memset(ebias, EXP_BIAS)

            bias_tiles = {}
            if use_bq:
                bq_sb = ws.tile([128, 2], F32, tag="bq")
                nc.sync.dma_start(out=bq_sb, in_=bqP[:, :])
                bias_tiles["q"] = bq_sb
            if use_bk:
                bk_sb = ws.tile([128, 2], F32, tag="bk")
                nc.sync.dma_start(out=bk_sb, in_=bkP[:, :])
                bias_tiles["k"] = bk_sb
            if use_bv:
                bv_sb = ws.tile([128, MPC], F32, tag="bv")
                nc.sync.dma_start(out=bv_sb, in_=bvR.ap().to_broadcast([128, MPC]))
                bias_tiles["v"] = bv_sb

            def emit_qk_proj(name, x_sb, w_sb, dst, mts):
                for mt in mts:
                    for c in range(NCH):
                        ps = ps_o.tile([128, CH], F32, tag="o", name=f"pj_{name}")
                        for kc in range(NKC):
                            nc.tensor.matmul(
                                ps,
                                lhsT=w_sb[:, kc, mt * 128 : (mt + 1) * 128],
                                rhs=x_sb[:, kc, c * CH : (c + 1) * CH],
                                start=(kc == 0),
                                stop=(kc == NKC - 1),
                            )
                        if name in bias_tiles:
                            nc.vector.tensor_scalar_add(
                                out=dst[:, mt, c * CH : (c + 1) * CH],
                                in0=ps,
                                scalar1=bias_tiles[name][:, mt : mt + 1],
                            )
                        else:
                            nc.scalar.copy(
                                out=dst[:, mt, c * CH : (c + 1) * CH], in_=ps
                            )

            def emit_v_lkt(x_sb, lkt):
                ps = ps_o.tile([128, MPC], F32, tag="o", name="pj_v")
                for kc in range(NKC):
                    nc.tensor.matmul(
                        ps,
                        lhsT=x_sb[:, kc, lkt * 128 : (lkt + 1) * 128],
                        rhs=wv_sb[:, kc, :],
                        start=(kc == 0),
                        stop=(kc == NKC - 1),
                    )
                psr = ps.rearrange("p (h d) -> p h d", h=HPC)
                if "v" in bias_tiles:
                    nc.vector.tensor_add(
                        out=Vaug[:, lkt, :, 0:64],
                        in0=psr,
                        in1=bias_tiles["v"].rearrange("p (h d) -> p h d", h=HPC),
                    )
                else:
                    nc.scalar.copy(out=Vaug[:, lkt, :, 0:64], in_=psr)

            x_q = xs.tile([128, NKC, L], BF16, tag="x", name="x_q")
            for kc in range(NKC):
                nc.sync.dma_start(out=x_q[:, kc, :], in_=qT[:, kc, :])
            emit_qk_proj("q", x_q, wq_sb, QT, (0, 1))
            x_k = xs.tile([128, NKC, L], BF16, tag="x", name="x_k")
            for kc in range(NKC):
                nc.sync.dma_start(out=x_k[:, kc, :], in_=kT[:, kc, :])
            emit_qk_proj("k", x_k, wk_sb, KT, (0, 1))
            x_v = xs.tile([128, NKC, L], BF16, tag="x", name="x_v")
            for kc in range(NKC):
                nc.sync.dma_start(out=x_v[:, kc, :], in_=vT[:, kc, :])

            OT = opl.tile([128, 2, L], BF16, tag="ot")

            def emit_phase_b(pr, v_x=None):
                # Even/odd heads sit at partition bases 0/64, so their K=64
                # S-matmuls pack into different PE row-groups (concurrent).
                for half in range(2):
                    o_ps = {}
                    for hp in range(2):
                        o_ps[hp] = ps_o.tile(
                            [128, 1024], F32, tag="o", name=f"o_{pr}_{half}_{hp}"
                        )
                    for lkt in range(NLKT):
                        g_sb = gp.tile([128, L], BF16, tag="g")
                        for hp in range(2):
                            nc.sync.dma_start(
                                out=g_sb[:, hp * HF : (hp + 1) * HF],
                                in_=gT[
                                    2 * pr + hp,
                                    lkt * 128 : (lkt + 1) * 128,
                                    half * HF : (half + 1) * HF,
                                ],
                            )
                        tmp = tp.tile([128, L], F32, tag="tmp")
                        p_sb = pp.tile([128, L], BF16, tag="p")
                        s_w = ps_s.tile([128, L], F32, tag="s", name="s_att")
                        for c in range(2):
                            for hp in range(2):
                                nc.tensor.matmul(
                                    s_w[
                                        :, hp * HF + c * CH : hp * HF + (c + 1) * CH
                                    ],
                                    lhsT=KT[
                                        hp * 64 : hp * 64 + 64,
                                        pr,
                                        lkt * 128 : (lkt + 1) * 128,
                                    ],
                                    rhs=QT[
                                        hp * 64 : hp * 64 + 64,
                                        pr,
                                        half * HF + c * CH : half * HF + (c + 1) * CH,
                                    ],
                                    start=True,
                                    stop=True,
                                )
                        nc.vector.tensor_mul(out=tmp, in0=s_w, in1=g_sb)
                        nc.scalar.activation(
                            out=p_sb, in_=tmp, func=AF.Exp, bias=ebias, scale=1.0
                        )
                        if masked:
                            mb_sb = gp.tile([128, HF], BF16, tag="mb")
                            nc.sync.dma_start(
                                out=mb_sb,
                                in_=mbT[
                                    lkt * 128 : (lkt + 1) * 128,
                                    half * HF : (half + 1) * HF,
                                ],
                            )
                            for hp in range(2):
                                nc.vector.tensor_mul(
                                    out=p_sb[:, hp * HF : (hp + 1) * HF],
                                    in0=p_sb[:, hp * HF : (hp + 1) * HF],
                                    in1=mb_sb,
                                )
                        for c in range(2):
                            for hp in range(2):
                                nc.tensor.matmul(
                                    o_ps[hp][:, c * CH : (c + 1) * CH],
                                    lhsT=Vaug[:, lkt, 2 * pr + hp, :],
                                    rhs=p_sb[
                                        :, hp * HF + c * CH : hp * HF + (c + 1) * CH
                                    ],
                                    start=(lkt == 0),
                                    stop=(lkt == NLKT - 1),
                                )
                    for hp in range(2):
                        d_sb = rp.tile([64, 1024], F32, tag="d")
                        nc.scalar.copy(out=d_sb, in_=o_ps[hp][64:128, :])
                        r_sb = rp.tile([64, 1024], F32, tag="r")
                        nc.vector.reciprocal_approx_fast(r_sb, d_sb)
                        nc.vector.tensor_mul(
                            out=OT[
                                hp * 64 : hp * 64 + 64,
                                pr,
                                half * HF : (half + 1) * HF,
                            ],
                            in0=o_ps[hp][0:64, :],
                            in1=r_sb,
                        )
                    nc.sync.dma_start(
                        out=oT[:, pr, half * HF : (half + 1) * HF],
                        in_=OT[:, pr, half * HF : (half + 1) * HF],
                    )

            for lkt in range(NLKT):
                emit_v_lkt(x_v, lkt)
            emit_phase_b(0)
            emit_phase_b(1)

    nc.finalize()
    return nc


def build_l2(use_bo: bool, use_gamma: bool, use_beta: bool):
    nc = bacc.Bacc("TRN2", target_bir_lowering=False)

    oTf = nc.declare_dram_parameter("oTf", [128, NKC, CH], BF16, isOutput=False)
    woTs = nc.declare_dram_parameter("woTs", [128, NKC, D], BF16, isOutput=False)
    qres = nc.declare_dram_parameter("qres", [4, 128, D], F32, isOutput=False)
    if use_bo:
        boR = nc.declare_dram_parameter("boR", [1, D], F32, isOutput=False)
    if use_gamma:
        gaR = nc.declare_dram_parameter("gaR", [1, D], F32, isOutput=False)
    if use_beta:
        beR = nc.declare_dram_parameter("beR", [1, D], F32, isOutput=False)
    yout = nc.declare_dram_parameter("yout", [4, 128, D], F32, isOutput=True)

    with tile.TileContext(nc) as tc:
        with (
            tc.tile_pool(name="ins", bufs=1) as ins,
            tc.tile_pool(name="res", bufs=4) as res,
            tc.tile_pool(name="xb", bufs=3) as xb,
            tc.tile_pool(name="st", bufs=3) as st,
            tc.tile_pool(name="ps", bufs=4, space="PSUM") as psp,
        ):
            oT_sb = ins.tile([128, NKC, CH], BF16, tag="ot")
            wo_sb = ins.tile([128, NKC, D], BF16, tag="wo")
            for kc in range(NKC):
                nc.sync.dma_start(out=oT_sb[:, kc, :], in_=oTf[:, kc, :])
                nc.sync.dma_start(out=wo_sb[:, kc, :], in_=woTs[:, kc, :])
            eps_sb = ins.tile([128, 1], F32, tag="eps")
            nc.vector.memset(eps_sb, EPS)
            bo_sb = ga_sb = be_sb = None
            if use_bo:
                bo_sb = ins.tile([128, D], F32, tag="bo")
                nc.sync.dma_start(out=bo_sb, in_=boR.ap().to_broadcast([128, D]))
            if use_gamma:
                ga_sb = ins.tile([128, D], F32, tag="ga")
                nc.sync.dma_start(out=ga_sb, in_=gaR.ap().to_broadcast([128, D]))
            if use_beta:
                be_sb = ins.tile([128, D], F32, tag="be")
                nc.sync.dma_start(out=be_sb, in_=beR.ap().to_broadcast([128, D]))

            fused_ln = bo_sb is None

            for m in range(4):
                q_sb = res.tile([128, D], F32, tag="q")
                nc.sync.dma_start(out=q_sb, in_=qres[m, :, :])
                x = xb.tile([128, D], F32, tag="x")
                accs = st.tile([128, 2], F32, tag="accs")
                for n in range(2):
                    ps = psp.tile([128, 512], F32, tag="mm")
                    for kc in range(NKC):
                        nc.tensor.matmul(
                            ps,
                            lhsT=oT_sb[:, kc, m * 128 : (m + 1) * 128],
                            rhs=wo_sb[:, kc, n * 512 : (n + 1) * 512],
                            start=(kc == 0),
                            stop=(kc == NKC - 1),
                        )
                    if fused_ln:
                        # x = fc + residual, and accumulate the row-sum
                        nc.vector.scalar_tensor_tensor(
                            out=x[:, n * 512 : (n + 1) * 512],
                            in0=ps,
                            scalar=1.0,
                            in1=q_sb[:, n * 512 : (n + 1) * 512],
                            op0=mybir.AluOpType.mult,
                            op1=mybir.AluOpType.add,
                            accum_out=accs[:, n : n + 1],
                        )
                    else:
                        nc.vector.tensor_add(
                            out=x[:, n * 512 : (n + 1) * 512],
                            in0=ps,
                            in1=q_sb[:, n * 512 : (n + 1) * 512],
                        )
                if fused_ln:
                    # variance via ACT: ssq = sum(x^2) (Square writes a scratch
                    # we ignore); mean/var assembled from the two accumulators
                    scr = xb.tile([128, D], F32, tag="scr")
                    ssq = st.tile([128, 1], F32, tag="ssq")
                    nc.scalar.activation(
                        out=scr, in_=x, func=AF.Square, accum_out=ssq
                    )
                    mu = st.tile([128, 1], F32, tag="mu")
                    nc.vector.tensor_scalar(
                        out=mu,
                        in0=accs[:, 0:1],
                        scalar1=accs[:, 1:2],
                        scalar2=1.0 / D,
                        op0=mybir.AluOpType.add,
                        op1=mybir.AluOpType.mult,
                    )
                    musq = st.tile([128, 1], F32, tag="musq")
                    nc.vector.tensor_mul(out=musq, in0=mu, in1=mu)
                    var = st.tile([128, 1], F32, tag="var")
                    nc.vector.tensor_scalar(
                        out=var,
                        in0=ssq,
                        scalar1=1.0 / D,
                        scalar2=musq,
                        op0=mybir.AluOpType.mult,
                        op1=mybir.AluOpType.subtract,
                    )
                    std = st.tile([128, 1], F32, tag="std")
                    nc.scalar.activation(
                        out=std, in_=var, func=AF.Sqrt, bias=eps_sb, scale=1.0
                    )
                else:
                    if bo_sb is not None:
                        nc.vector.tensor_add(out=x, in0=x, in1=bo_sb)
                    stats = st.tile([128, 2, 6], F32, tag="stats")
                    for half in range(2):
                        nc.vector.bn_stats(
                            out=stats[:, half, :],
                            in_=x[:, half * 512 : (half + 1) * 512],
                        )
                    mv = st.tile([128, 2], F32, tag="mv")
                    nc.vector.bn_aggr(out=mv, in_=stats)
                    mu = mv[:, 0:1]
                    std = st.tile([128, 1], F32, tag="std")
                    nc.scalar.activation(
                        out=std, in_=mv[:, 1:2], func=AF.Sqrt, bias=eps_sb, scale=1.0
                    )
                rstd = st.tile([128, 1], F32, tag="rstd")
                nc.vector.reciprocal(out=rstd, in_=std)
                y = xb.tile([128, D], F32, tag="y")
                nc.vector.tensor_scalar(
                    out=y,
                    in0=x,
                    scalar1=mu,
                    scalar2=rstd,
                    op0=mybir.AluOpType.subtract,
                    op1=mybir.AluOpType.mult,
                )
                if ga_sb is not None:
                    nc.vector.tensor_mul(out=y, in0=y, in1=ga_sb)
                if be_sb is not None:
                    nc.vector.tensor_add(out=y, in0=y, in1=be_sb)
                nc.sync.dma_start(out=yout[m, :, :], in_=y)

    nc.finalize()
    return nc


_L1_CACHE = {}
_L2_CACHE = {}
LAST_RUNS = []  # (tag, nc, in_maps) of the most recent kernel() call, for profiling


def kernel(
    q, k, v, k_gate, mask, wq, bq, wk, bk, wv, bv, wo, bo, gamma, beta
):
    q = np.asarray(q, np.float32)
    k = np.asarray(k, np.float32)
    v = np.asarray(v, np.float32)
    k_gate = np.asarray(k_gate, np.float32)
    mask = np.asarray(mask)
    wq = np.asarray(wq, np.float32)
    wk = np.asarray(wk, np.float32)
    wv = np.asarray(wv, np.float32)
    wo = np.asarray(wo, np.float32)
    bq = np.asarray(bq, np.float32)
    bk = np.asarray(bk, np.float32)
    bv = np.asarray(bv, np.float32)
    bo = np.asarray(bo, np.float32)
    gamma = np.asarray(gamma, np.float32)
    beta = np.asarray(beta, np.float32)

    masked = bool(mask.any())
    use_bq = bool(np.any(bq))
    use_bk = bool(np.any(bk))
    use_bv = bool(np.any(bv))
    use_bo = bool(np.any(bo))
    use_gamma = bool(np.any(gamma != 1.0))
    use_beta = bool(np.any(beta))

    temp = float(np.float32(np.power(DK, 0.5)))

    key1 = (masked, use_bq, use_bk, use_bv)
    if key1 not in _L1_CACHE:
        _L1_CACHE[key1] = build_l1(*key1)
    nc1 = _L1_CACHE[key1]

    # ---- stage launch-1 inputs ----
    xT = {}  # (name, b) -> [128, NKC, L] bf16
    for b in range(B):
        xT[("q", b)] = _bf(_kc_layout(q[b].T))
        xT[("k", b)] = _bf(_kc_layout(k[b].T))
        xT[("v", b)] = _bf(_kc_layout(v[b].T))
    wts = {}  # (name, hg) -> [128, NKC, MPC] bf16
    for hg in range(4):
        sl = slice(hg * MPC, (hg + 1) * MPC)
        wts[("q", hg)] = _bf(_kc_layout(wq[sl].T / temp))
        wts[("k", hg)] = _bf(_kc_layout(wk[sl].T))
        wts[("v", hg)] = _bf(_kc_layout(wv[sl].T))

    in_maps = []
    for c in range(NCORE):
        b, hg = c // 4, c % 4
        hsl = slice(hg * HPC, (hg + 1) * HPC)
        m = {
            "qT": xT[("q", b)],
            "kT": xT[("k", b)],
            "vT": xT[("v", b)],
            "wqT": wts[("q", hg)],
            "wkT": wts[("k", hg)],
            "wvT": wts[("v", hg)],
            "gT": _bf(k_gate[b, hsl].transpose(0, 2, 1)),
        }
        if use_bq:
            m["bqP"] = np.ascontiguousarray(
                (bq[hg * MPC : (hg + 1) * MPC] / temp).reshape(2, 128).T
            )
        if use_bk:
            m["bkP"] = np.ascontiguousarray(
                bk[hg * MPC : (hg + 1) * MPC].reshape(2, 128).T
            )
        if use_bv:
            m["bvR"] = bv[hg * MPC : (hg + 1) * MPC].reshape(1, MPC).copy()
        if masked:
            m["mbT"] = _bf((~mask[b]).astype(np.float32).T)
        in_maps.append(m)

    LAST_RUNS.clear()
    LAST_RUNS.append(("L1", nc1, in_maps))
    res1 = run_bass_kernel_spmd(nc1, in_maps, list(range(NCORE)))

    # assemble O^T per batch: [H*DV, L] bf16
    OTb = []
    for b in range(B):
        parts = []
        for hg in range(4):
            r = res1.results[b * 4 + hg]["oT"]  # [128, 2, L] bf16
            parts.append(np.ascontiguousarray(r.transpose(1, 0, 2)).reshape(MPC, L))
        OTb.append(np.concatenate(parts, axis=0))  # [1024, L]

    key2 = (use_bo, use_gamma, use_beta)
    if key2 not in _L2_CACHE:
        _L2_CACHE[key2] = build_l2(*key2)
    nc2 = _L2_CACHE[key2]

    woTs = _bf(_kc_layout(wo.T))
    in_maps2 = []
    for c in range(NCORE):
        b, rchunk = c // 4, c % 4
        rows = slice(rchunk * CH, (rchunk + 1) * CH)
        otf = OTb[b][:, rows]  # [1024, 512] bf16
        m = {
            "oTf": np.ascontiguousarray(
                otf.reshape(NKC, 128, CH).transpose(1, 0, 2)
            ),
            "woTs": woTs,
            "qres": np.ascontiguousarray(q[b, rows].reshape(4, 128, D)),
        }
        if use_bo:
            m["boR"] = bo.reshape(1, D).copy()
        if use_gamma:
            m["gaR"] = gamma.reshape(1, D).copy()
        if use_beta:
            m["beR"] = beta.reshape(1, D).copy()
        in_maps2.append(m)

    LAST_RUNS.append(("L2", nc2, in_maps2))
    res2 = run_bass_kernel_spmd(nc2, in_maps2, list(range(NCORE)))

    out = np.empty((B, L, D), np.float32)
    for c in range(NCORE):
        b, rchunk = c // 4, c % 4
        out[b, rchunk * CH : (rchunk + 1) * CH] = res2.results[c]["yout"].reshape(
            CH, D
        )
    return out



# revision 8
# speedup vs baseline: 1.2877x; 1.2877x over previous
"""Trainium2 Bass kernel for gated multi-head attention + residual + LayerNorm.

Problem (nn_CNP_5669356834854):
    B=2, L=2048, D=1024, H=16, DK=DV=64
    Q = q@wq.T+bq; K = k@wk.T+bk; V = v@wv.T+bv   (per-head split)
    attn = softmax((Q K^T / sqrt(DK)) * k_gate  [masked])
    out = LayerNorm(attn @ V @ wo.T + bo + q)

Sharding: 8 cores = (batch b in {0,1}) x (head-group hg in {0..3}, 4 heads each).
Launch 1 computes UNNORMALIZED per-head attention outputs O^T plus the softmax
denominators (via the ones-augmented V trick).  Launch 2 shards
(batch, 512-row chunk): it normalizes (one bf16 2x-mode multiply against
host-expanded reciprocal denominators), then does output projection +
residual + LayerNorm.

Everything is computed in "T-space" (transposed layouts) so that no on-chip
transposes are needed:
    S^T[lk,lq] = matmul(lhsT=K^T tile, rhs=Q^T)
    P^T = exp(S^T * gate^T - 20)        (the -20 cancels in normalization)
    O_aug = [V | ones64]^T-matmul: rows 0:64 = unnormalized O^T, rows 64:128 =
            the denominator replicated across 64 partitions (free broadcast).

L1 pipeline design: S-tiles are per-head [128,1024] in a 2-slot PSUM ring
(4 banks) so the next head's S-matmuls overlap the current gate-multiply;
O-matmuls are emitted 2 iterations behind (software pipelining) so the PE
never sits in front of the exp dependency.  The gate arrives as host-packed
contiguous 512KB slabs (one DMA per (pr,half,lkt)) for near-peak HBM rate.
"""

import numpy as np
import ml_dtypes

import concourse.bacc as bacc
import concourse.tile as tile
from concourse import mybir
from concourse.bass_utils import run_bass_kernel_spmd

B, L, D, H, DK, DV = 2, 2048, 1024, 16, 64, 64
EPS = 1e-5
NCORE = 8
HPC = 4  # heads per core
NKC = D // 128  # 8 contraction chunks
NLKT = L // 128  # 16 lk tiles
CH = 512  # L2 row-chunk per core
MPC = HPC * DK  # 256 projected rows per core
HF = L // 2  # 1024, lq per (pr, half) iteration
EXP_BIAS = -20.0

F32 = mybir.dt.float32
BF16 = mybir.dt.bfloat16
NPBF16 = ml_dtypes.bfloat16
AF = mybir.ActivationFunctionType

PIPE = 2  # O-matmul emission lag (software pipeline depth)


def _bf(x):
    return np.ascontiguousarray(x).astype(NPBF16)


def _kc_layout(a):
    """[D, N] -> [128, NKC, N] with row r = kc*128+p  ->  [p, kc, :]."""
    d, n = a.shape
    assert d == NKC * 128
    return np.ascontiguousarray(a.reshape(NKC, 128, n).transpose(1, 0, 2))


def build_l1(masked: bool, use_bq: bool, use_bk: bool, use_bv: bool):
    nc = bacc.Bacc("TRN2", target_bir_lowering=False)

    qT = nc.declare_dram_parameter("qT", [128, NKC, L], BF16, isOutput=False)
    kT = nc.declare_dram_parameter("kT", [128, NKC, L], BF16, isOutput=False)
    vT = nc.declare_dram_parameter("vT", [128, NKC, L], BF16, isOutput=False)
    wqT = nc.declare_dram_parameter("wqT", [128, NKC, MPC], BF16, isOutput=False)
    wkT = nc.declare_dram_parameter("wkT", [128, NKC, MPC], BF16, isOutput=False)
    wvT = nc.declare_dram_parameter("wvT", [128, NKC, MPC], BF16, isOutput=False)
    # host-packed gate: gPK[pr, half, lkt, p, hp*1024 + c*512 + i]
    gPK = nc.declare_dram_parameter(
        "gPK", [2, 2, NLKT, 128, 2 * HF], BF16, isOutput=False
    )
    if use_bq:
        bqP = nc.declare_dram_parameter("bqP", [128, 2], F32, isOutput=False)
    if use_bk:
        bkP = nc.declare_dram_parameter("bkP", [128, 2], F32, isOutput=False)
    if use_bv:
        bvR = nc.declare_dram_parameter("bvR", [1, MPC], F32, isOutput=False)
    if masked:
        mbT = nc.declare_dram_parameter("mbT", [L, L], BF16, isOutput=False)
    # unnormalized O (rows 0:64 per hp) + denominator (row 64)
    oU = nc.declare_dram_parameter("oU", [2, 2, 65, 2, HF], BF16, isOutput=True)

    with tile.TileContext(nc) as tc:
        with (
            tc.tile_pool(name="xs", bufs=2) as xs,
            tc.tile_pool(name="ws", bufs=1) as ws,
            tc.tile_pool(name="qk", bufs=1) as qk,
            tc.tile_pool(name="gp", bufs=6) as gp,
            tc.tile_pool(name="tp", bufs=3) as tp,
            tc.tile_pool(name="pp", bufs=4) as pp,
            tc.tile_pool(name="op", bufs=2) as opl,
            tc.tile_pool(name="ps_s", bufs=2, space="PSUM") as ps_s,
            tc.tile_pool(name="ps_o", bufs=2, space="PSUM") as ps_o,
        ):
            wq_sb = ws.tile([128, NKC, MPC], BF16, tag="wq")
            nc.sync.dma_start(out=wq_sb, in_=wqT[:, :, :])
            wk_sb = ws.tile([128, NKC, MPC], BF16, tag="wk")
            nc.sync.dma_start(out=wk_sb, in_=wkT[:, :, :])
            wv_sb = ws.tile([128, NKC, MPC], BF16, tag="wv")
            nc.sync.dma_start(out=wv_sb, in_=wvT[:, :, :])

            QT = qk.tile([128, 2, L], BF16, tag="qt")
            KT = qk.tile([128, 2, L], BF16, tag="kt")
            Vaug = qk.tile([128, NLKT, HPC, 128], BF16, tag="va")
            nc.vector.memset(Vaug[:, :, :, 64:128], 1.0)
            ebias = ws.tile([128, 1], F32, tag="eb")
            nc.vector.memset(ebias, EXP_BIAS)

            bias_tiles = {}
            if use_bq:
                bq_sb = ws.tile([128, 2], F32, tag="bq")
                nc.sync.dma_start(out=bq_sb, in_=bqP[:, :])
                bias_tiles["q"] = bq_sb
            if use_bk:
                bk_sb = ws.tile([128, 2], F32, tag="bk")
                nc.sync.dma_start(out=bk_sb, in_=bkP[:, :])
                bias_tiles["k"] = bk_sb
            if use_bv:
                bv_sb = ws.tile([128, MPC], F32, tag="bv")
                nc.sync.dma_start(out=bv_sb, in_=bvR.ap().to_broadcast([128, MPC]))
                bias_tiles["v"] = bv_sb

            def emit_v_lkt(x_sb, lkt):
                ps = ps_o.tile([128, MPC], F32, tag="o", name="pj_v")
                for kc in range(NKC):
                    nc.tensor.matmul(
                        ps,
                        lhsT=x_sb[:, kc, lkt * 128 : (lkt + 1) * 128],
                        rhs=wv_sb[:, kc, :],
                        start=(kc == 0),
                        stop=(kc == NKC - 1),
                    )
                psr = ps.rearrange("p (h d) -> p h d", h=HPC)
                if "v" in bias_tiles:
                    nc.vector.tensor_add(
                        out=Vaug[:, lkt, :, 0:64],
                        in0=psr,
                        in1=bias_tiles["v"].rearrange("p (h d) -> p h d", h=HPC),
                    )
                else:
                    nc.scalar.copy(out=Vaug[:, lkt, :, 0:64], in_=psr)

            # ---- projections ----
            x_q = xs.tile([128, NKC, L], BF16, tag="x", name="x_q")
            for kc in range(NKC):
                nc.sync.dma_start(out=x_q[:, kc, :], in_=qT[:, kc, :])
            x_k = xs.tile([128, NKC, L], BF16, tag="x", name="x_k")
            for kc in range(NKC):
                nc.sync.dma_start(out=x_k[:, kc, :], in_=kT[:, kc, :])
            x_v = xs.tile([128, NKC, L], BF16, tag="x", name="x_v")
            for kc in range(NKC):
                nc.sync.dma_start(out=x_v[:, kc, :], in_=vT[:, kc, :])

            # Q/K projections: for each mt (=pr) block we need ALL lq columns.
            # Two [128,1024] psums per (name, mt): cols (0:1024) and (1024:2048).
            def emit_qk_full(name, x_sb, w_sb, dst):
                for mt in range(2):
                    for lqh in range(2):
                        ps = ps_s.tile(
                            [128, 2 * 512], F32, tag="s", name=f"pj_{name}{mt}{lqh}"
                        )
                        for c in range(2):
                            lo = lqh * 1024 + c * 512
                            for kc in range(NKC):
                                nc.tensor.matmul(
                                    ps[:, c * 512 : (c + 1) * 512],
                                    lhsT=w_sb[:, kc, mt * 128 : (mt + 1) * 128],
                                    rhs=x_sb[:, kc, lo : lo + 512],
                                    start=(kc == 0),
                                    stop=(kc == NKC - 1),
                                )
                        if name in bias_tiles:
                            nc.vector.tensor_scalar_add(
                                out=dst[:, mt, lqh * 1024 : (lqh + 1) * 1024],
                                in0=ps,
                                scalar1=bias_tiles[name][:, mt : mt + 1],
                            )
                        else:
                            nc.scalar.copy(
                                out=dst[:, mt, lqh * 1024 : (lqh + 1) * 1024],
                                in_=ps,
                            )

            emit_qk_full("q", x_q, wq_sb, QT)
            emit_qk_full("k", x_k, wk_sb, KT)
            for lkt in range(NLKT):
                emit_v_lkt(x_v, lkt)

            # ---- attention ----
            def attention_block(pr, half):
                o_ps = {}
                for hp in range(2):
                    o_ps[hp] = ps_o.tile(
                        [128, HF], F32, tag="o", name=f"o_{pr}_{half}_{hp}"
                    )
                p_tiles = {}

                def emit_front(k):
                    g_sb = gp.tile([128, 2 * HF], BF16, tag="g")
                    nc.sync.dma_start(out=g_sb, in_=gPK[pr, half, k, :, :])
                    tmp = tp.tile([128, 2 * HF], BF16, tag="tmp")
                    for hp in range(2):
                        s_w = ps_s.tile(
                            [128, HF], F32, tag="s", name=f"s_{pr}_{half}_{k}_{hp}"
                        )
                        for c in range(2):
                            nc.tensor.matmul(
                                s_w[:, c * 512 : (c + 1) * 512],
                                lhsT=KT[
                                    hp * 64 : hp * 64 + 64,
                                    pr,
                                    k * 128 : (k + 1) * 128,
                                ],
                                rhs=QT[
                                    hp * 64 : hp * 64 + 64,
                                    pr,
                                    half * HF + c * 512 : half * HF + (c + 1) * 512,
                                ],
                                start=True,
                                stop=True,
                            )
                        nc.vector.tensor_mul(
                            out=tmp[:, hp * HF : (hp + 1) * HF],
                            in0=s_w,
                            in1=g_sb[:, hp * HF : (hp + 1) * HF],
                        )
                    p_sb = pp.tile([128, 2 * HF], BF16, tag="p")
                    nc.scalar.activation(
                        out=p_sb, in_=tmp, func=AF.Exp, bias=ebias, scale=1.0
                    )
                    if masked:
                        mb_sb = gp.tile([128, HF], BF16, tag="mb")
                        nc.sync.dma_start(
                            out=mb_sb,
                            in_=mbT[
                                k * 128 : (k + 1) * 128,
                                half * HF : (half + 1) * HF,
                            ],
                        )
                        for hp in range(2):
                            nc.vector.tensor_mul(
                                out=p_sb[:, hp * HF : (hp + 1) * HF],
                                in0=p_sb[:, hp * HF : (hp + 1) * HF],
                                in1=mb_sb,
                            )
                    p_tiles[k] = p_sb

                def emit_back(k):
                    p_sb = p_tiles.pop(k)
                    for hp in range(2):
                        for c in range(2):
                            nc.tensor.matmul(
                                o_ps[hp][:, c * 512 : (c + 1) * 512],
                                lhsT=Vaug[:, k, 2 * pr + hp, :],
                                rhs=p_sb[
                                    :, hp * HF + c * 512 : hp * HF + (c + 1) * 512
                                ],
                                start=(k == 0),
                                stop=(k == NLKT - 1),
                            )

                for k in range(NLKT + PIPE):
                    if k < NLKT:
                        emit_front(k)
                    if k >= PIPE:
                        emit_back(k - PIPE)

                OUa = opl.tile([65, 2, HF], BF16, tag="ou")
                for hp in range(2):
                    nc.scalar.copy(out=OUa[:, hp, :], in_=o_ps[hp][0:65, :])
                nc.sync.dma_start(out=oU[pr, half, :, :, :], in_=OUa)

            for pr in range(2):
                for half in range(2):
                    attention_block(pr, half)

    nc.finalize()
    return nc


def build_l2(use_bo: bool, use_gamma: bool, use_beta: bool):
    nc = bacc.Bacc("TRN2", target_bir_lowering=False)

    oTf = nc.declare_dram_parameter("oTf", [128, NKC, CH], BF16, isOutput=False)
    rdK = nc.declare_dram_parameter("rdK", [128, NKC, CH], BF16, isOutput=False)
    woTs = nc.declare_dram_parameter("woTs", [128, NKC, D], BF16, isOutput=False)
    qres = nc.declare_dram_parameter("qres", [4, 128, D], BF16, isOutput=False)
    if use_bo:
        boR = nc.declare_dram_parameter("boR", [1, D], F32, isOutput=False)
    if use_gamma:
        gaR = nc.declare_dram_parameter("gaR", [1, D], F32, isOutput=False)
    if use_beta:
        beR = nc.declare_dram_parameter("beR", [1, D], F32, isOutput=False)
    yout = nc.declare_dram_parameter("yout", [4, 128, D], F32, isOutput=True)

    with tile.TileContext(nc) as tc:
        with (
            tc.tile_pool(name="ins", bufs=1) as ins,
            tc.tile_pool(name="res", bufs=4) as res,
            tc.tile_pool(name="xb", bufs=4) as xb,
            tc.tile_pool(name="st", bufs=4) as st,
            tc.tile_pool(name="ps", bufs=6, space="PSUM") as psp,
        ):
            oT_sb = ins.tile([128, NKC, CH], BF16, tag="ot")
            rd_sb = ins.tile([128, NKC, CH], BF16, tag="rd")
            nc.sync.dma_start(out=oT_sb, in_=oTf[:, :, :])
            nc.sync.dma_start(out=rd_sb, in_=rdK[:, :, :])
            wo_sb = ins.tile([128, NKC, D], BF16, tag="wo")
            for kc in range(NKC):
                nc.sync.dma_start(out=wo_sb[:, kc, :], in_=woTs[:, kc, :])
            eps_sb = ins.tile([128, 1], F32, tag="eps")
            nc.vector.memset(eps_sb, EPS)
            bo_sb = ga_sb = be_sb = None
            if use_bo:
                bo_sb = ins.tile([128, D], F32, tag="bo")
                nc.sync.dma_start(out=bo_sb, in_=boR.ap().to_broadcast([128, D]))
            if use_gamma:
                ga_sb = ins.tile([128, D], F32, tag="ga")
                nc.sync.dma_start(out=ga_sb, in_=gaR.ap().to_broadcast([128, D]))
            if use_beta:
                be_sb = ins.tile([128, D], F32, tag="be")
                nc.sync.dma_start(out=be_sb, in_=beR.ap().to_broadcast([128, D]))

            # normalize: oTn = O_un * (1/den), bf16 2x-mode, one instruction
            oTn = ins.tile([128, NKC, CH], BF16, tag="on")
            nc.vector.tensor_mul(out=oTn, in0=oT_sb, in1=rd_sb)

            q_tiles = []
            for m in range(4):
                q_sb = res.tile([128, D], BF16, tag="q", name=f"q{m}")
                nc.sync.dma_start(out=q_sb, in_=qres[m, :, :])
                q_tiles.append(q_sb)

            fused_ln = bo_sb is None

            # emit all matmuls densely (PE warm), LN chains trail per-m
            ps_mn = {}
            for m in range(4):
                for n in range(2):
                    ps = psp.tile([128, 512], F32, tag="mm", name=f"mm{m}{n}")
                    for kc in range(NKC):
                        nc.tensor.matmul(
                            ps,
                            lhsT=oTn[:, kc, m * 128 : (m + 1) * 128],
                            rhs=wo_sb[:, kc, n * 512 : (n + 1) * 512],
                            start=(kc == 0),
                            stop=(kc == NKC - 1),
                        )
                    ps_mn[(m, n)] = ps

                q_sb = q_tiles[m]
                x = xb.tile([128, D], F32, tag="x")
                accs = st.tile([128, 2], F32, tag="accs")
                for n in range(2):
                    ps = ps_mn.pop((m, n))
                    if fused_ln:
                        nc.vector.scalar_tensor_tensor(
                            out=x[:, n * 512 : (n + 1) * 512],
                            in0=ps,
                            scalar=1.0,
                            in1=q_sb[:, n * 512 : (n + 1) * 512],
                            op0=mybir.AluOpType.mult,
                            op1=mybir.AluOpType.add,
                            accum_out=accs[:, n : n + 1],
                        )
                    else:
                        nc.vector.tensor_add(
                            out=x[:, n * 512 : (n + 1) * 512],
                            in0=ps,
                            in1=q_sb[:, n * 512 : (n + 1) * 512],
                        )
                if fused_ln:
                    scr = xb.tile([128, D], F32, tag="scr")
                    ssq = st.tile([128, 1], F32, tag="ssq")
                    nc.scalar.activation(
                        out=scr, in_=x, func=AF.Square, accum_out=ssq
                    )
                    mu = st.tile([128, 1], F32, tag="mu")
                    nc.vector.tensor_scalar(
                        out=mu,
                        in0=accs[:, 0:1],
                        scalar1=accs[:, 1:2],
                        scalar2=1.0 / D,
                        op0=mybir.AluOpType.add,
                        op1=mybir.AluOpType.mult,
                    )
                    musq = st.tile([128, 1], F32, tag="musq")
                    nc.vector.tensor_mul(out=musq, in0=mu, in1=mu)
                    var = st.tile([128, 1], F32, tag="var")
                    nc.vector.tensor_scalar(
                        out=var,
                        in0=ssq,
                        scalar1=1.0 / D,
                        scalar2=musq,
                        op0=mybir.AluOpType.mult,
                        op1=mybir.AluOpType.subtract,
                    )
                    std = st.tile([128, 1], F32, tag="std")
                    nc.scalar.activation(
                        out=std, in_=var, func=AF.Sqrt, bias=eps_sb, scale=1.0
                    )
                else:
                    if bo_sb is not None:
                        nc.vector.tensor_add(out=x, in0=x, in1=bo_sb)
                    stats = st.tile([128, 2, 6], F32, tag="stats")
                    for hh in range(2):
                        nc.vector.bn_stats(
                            out=stats[:, hh, :],
                            in_=x[:, hh * 512 : (hh + 1) * 512],
                        )
                    mv = st.tile([128, 2], F32, tag="mv")
                    nc.vector.bn_aggr(out=mv, in_=stats)
                    mu = mv[:, 0:1]
                    std = st.tile([128, 1], F32, tag="std")
                    nc.scalar.activation(
                        out=std, in_=mv[:, 1:2], func=AF.Sqrt, bias=eps_sb, scale=1.0
                    )
                rstd = st.tile([128, 1], F32, tag="rstd")
                nc.vector.reciprocal(out=rstd, in_=std)
                y = xb.tile([128, D], F32, tag="y")
                nc.vector.tensor_scalar(
                    out=y,
                    in0=x,
                    scalar1=mu,
                    scalar2=rstd,
                    op0=mybir.AluOpType.subtract,
                    op1=mybir.AluOpType.mult,
                )
                if ga_sb is not None:
                    nc.vector.tensor_mul(out=y, in0=y, in1=ga_sb)
                if be_sb is not None:
                    nc.vector.tensor_add(out=y, in0=y, in1=be_sb)
                nc.sync.dma_start(out=yout[m, :, :], in_=y)

    nc.finalize()
    return nc


_L1_CACHE = {}
_L2_CACHE = {}
LAST_RUNS = []  # (tag, nc, in_maps) of the most recent kernel() call, for profiling


def kernel(
    q, k, v, k_gate, mask, wq, bq, wk, bk, wv, bv, wo, bo, gamma, beta
):
    q = np.asarray(q, np.float32)
    k = np.asarray(k, np.float32)
    v = np.asarray(v, np.float32)
    k_gate = np.asarray(k_gate, np.float32)
    mask = np.asarray(mask)
    wq = np.asarray(wq, np.float32)
    wk = np.asarray(wk, np.float32)
    wv = np.asarray(wv, np.float32)
    wo = np.asarray(wo, np.float32)
    bq = np.asarray(bq, np.float32)
    bk = np.asarray(bk, np.float32)
    bv = np.asarray(bv, np.float32)
    bo = np.asarray(bo, np.float32)
    gamma = np.asarray(gamma, np.float32)
    beta = np.asarray(beta, np.float32)

    masked = bool(mask.any())
    use_bq = bool(np.any(bq))
    use_bk = bool(np.any(bk))
    use_bv = bool(np.any(bv))
    use_bo = bool(np.any(bo))
    use_gamma = bool(np.any(gamma != 1.0))
    use_beta = bool(np.any(beta))

    temp = float(np.float32(np.power(DK, 0.5)))

    key1 = (masked, use_bq, use_bk, use_bv)
    if key1 not in _L1_CACHE:
        _L1_CACHE[key1] = build_l1(*key1)
    nc1 = _L1_CACHE[key1]

    # ---- stage launch-1 inputs ----
    xT = {}  # (name, b) -> [128, NKC, L] bf16
    for b in range(B):
        xT[("q", b)] = _bf(_kc_layout(q[b].T))
        xT[("k", b)] = _bf(_kc_layout(k[b].T))
        xT[("v", b)] = _bf(_kc_layout(v[b].T))
    wts = {}  # (name, hg) -> [128, NKC, MPC] bf16
    for hg in range(4):
        sl = slice(hg * MPC, (hg + 1) * MPC)
        wts[("q", hg)] = _bf(_kc_layout(wq[sl].T / temp))
        wts[("k", hg)] = _bf(_kc_layout(wk[sl].T))
        wts[("v", hg)] = _bf(_kc_layout(wv[sl].T))

    in_maps = []
    for c in range(NCORE):
        b, hg = c // 4, c % 4
        hsl = slice(hg * HPC, (hg + 1) * HPC)
        # gate pack: k_gate[b] is [head, lq, lk]; we need the transposed
        # per-tile layout gPK[pr, half, lkt, p, hp*1024 + i] = g[h, lq, lk]
        gh = k_gate[b, hsl]  # [4, 2048, 2048]  (head, lq, lk)
        gr = gh.reshape(2, 2, 2, HF, NLKT, 128)  # pr, hp, half, i, lkt, p
        gPK = _bf(gr.transpose(0, 2, 4, 5, 1, 3).reshape(2, 2, NLKT, 128, 2 * HF))
        m = {
            "qT": xT[("q", b)],
            "kT": xT[("k", b)],
            "vT": xT[("v", b)],
            "wqT": wts[("q", hg)],
            "wkT": wts[("k", hg)],
            "wvT": wts[("v", hg)],
            "gPK": gPK,
        }
        if use_bq:
            m["bqP"] = np.ascontiguousarray(
                (bq[hg * MPC : (hg + 1) * MPC] / temp).reshape(2, 128).T
            )
        if use_bk:
            m["bkP"] = np.ascontiguousarray(
                bk[hg * MPC : (hg + 1) * MPC].reshape(2, 128).T
            )
        if use_bv:
            m["bvR"] = bv[hg * MPC : (hg + 1) * MPC].reshape(1, MPC).copy()
        if masked:
            m["mbT"] = _bf((~mask[b]).astype(np.float32).T)
        in_maps.append(m)

    LAST_RUNS.clear()
    LAST_RUNS.append(("L1", nc1, in_maps))
    res1 = run_bass_kernel_spmd(nc1, in_maps, list(range(NCORE)))

    # assemble O_un^T per batch [1024, L] f32-ish (bf16 data) + denominators
    OTb = np.empty((B, H * DV, L), np.float32)
    DENb = np.empty((B, H, L), np.float32)
    for b in range(B):
        for hg in range(4):
            r = res1.results[b * 4 + hg]["oU"].astype(np.float32)
            # r: [pr, half, 65, hp, HF]
            for pr in range(2):
                for hp in range(2):
                    h = hg * 4 + 2 * pr + hp
                    blk = r[pr, :, :, hp, :]  # [half, 65, HF]
                    OTb[b, h * 64 : (h + 1) * 64, :] = np.concatenate(
                        [blk[0, :64], blk[1, :64]], axis=1
                    )
                    DENb[b, h, :HF] = blk[0, 64]
                    DENb[b, h, HF:] = blk[1, 64]

    key2 = (use_bo, use_gamma, use_beta)
    if key2 not in _L2_CACHE:
        _L2_CACHE[key2] = build_l2(*key2)
    nc2 = _L2_CACHE[key2]

    woTs = _bf(_kc_layout(wo.T))
    rd_full = 1.0 / DENb  # [B, H, L]
    in_maps2 = []
    for c in range(NCORE):
        b, rchunk = c // 4, c % 4
        rows = slice(rchunk * CH, (rchunk + 1) * CH)
        otf = OTb[b][:, rows]  # [1024, 512]
        # rdK[p, kc, i] = 1/den[head = 2*kc + p//64, row_i]
        rdc = rd_full[b][:, rows]  # [16, 512]
        rdK = np.empty((128, NKC, CH), np.float32)
        for kc in range(NKC):
            rdK[0:64, kc, :] = rdc[2 * kc]
            rdK[64:128, kc, :] = rdc[2 * kc + 1]
        m = {
            "oTf": _bf(otf.reshape(NKC, 128, CH).transpose(1, 0, 2)),
            "rdK": _bf(rdK),
            "woTs": woTs,
            "qres": _bf(q[b, rows].reshape(4, 128, D)),
        }
        if use_bo:
            m["boR"] = bo.reshape(1, D).copy()
        if use_gamma:
            m["gaR"] = gamma.reshape(1, D).copy()
        if use_beta:
            m["beR"] = beta.reshape(1, D).copy()
        in_maps2.append(m)

    LAST_RUNS.append(("L2", nc2, in_maps2))
    res2 = run_bass_kernel_spmd(nc2, in_maps2, list(range(NCORE)))

    out = np.empty((B, L, D), np.float32)
    for c in range(NCORE):
        b, rchunk = c // 4, c % 4
        out[b, rchunk * CH : (rchunk + 1) * CH] = res2.results[c]["yout"].reshape(
            CH, D
        )
    return out
